# revision 29
# baseline (speedup 1.0000x reference)
"""Trainium2 Bass kernel for a DynamicConv decoder layer.

Computation (fairseq DynamicConvDecoderLayer, eval mode, normalize_after):
    h  = x @ w1.T + b1                       # [T,B,E] -> [T,B,C]
    w  = softmax((h @ ww.T + bw) per-head)   # dynamic conv weights [T,B,H,K]
    c  = causal banded aggregation of h with per-position weights
    h2 = c @ w2.T + b2
    out = LayerNorm(x + h2) * gamma + beta

Distribution: data-parallel over batch (B=16 -> 2 per core on 8 cores).

Fast path (trivial bias/affine, the benchmarked configuration) uses
fp8-e4m3 DoubleRow matmuls with hi/lo error compensation:
  - Phase A (h1 = x @ w1T) and Phase B (conv logits from the host-fused
    weight (ww@w1)^T): 3-term compensated fp8 — (xhi+xlo)@whi + xhi@wlo —
    packed as DoubleRow pairs over E-chunk pairs (2 contraction chunks per
    PE instruction at 0.5 cyc/row -> 4x fewer PE-rows than bf16).
    Host precomputes the hi/lo fp8 splits and pair-interleaved layouts.
  - Softmax per (token, head) on ACT/DVE; weights cast to bf16.
  - Band build: GPSIMD local_scatter writes a per-head stacked band
    Band[p, h*128 + (p%64)+k] (64-token output blocks, zero-filled),
    one PE transpose per head gives Band^T[sigma, (blk0 tau | blk1 tau)].
  - Conv: per (head, 64-block) accumulating bf16 matmuls against h1
    token-tiles (94-token src windows; even blocks split across the
    previous/current h1 tile).
  - Phase D (h2 = conv @ w2T): conv^T cast to scaled fp8 on PSUM
    evacuation; 2-term compensation (ct8@w2hi + ct8@w2lo) in DoubleRow.
  - Residual + LN stats ride the PSUM evacuation (scalar_tensor_tensor
    with accum_out, ACT Square pass); rstd = exp(-0.5*ln(var+eps)); all
    ACT functions live in the single `natural_log_exp_and_others` table.

Non-trivial bias/affine inputs fall back to the legacy full-precision
(f32r/bf16) build.
"""

import sys
import os

sys.path.insert(0, "/opt/trn_rl_repo")

import numpy as np
from contextlib import ExitStack

import concourse.bass as bass
import concourse.bacc as bacc
import concourse.mybir as mybir
from concourse import tile

import ml_dtypes

T, B, E = 2048, 16, 1024
CDIM, H, KW = 1024, 16, 31
R = CDIM // H            # 64 channels per head
NB = 2                   # batch shard per core
NCORES = 8
P = 128
EPS = 1e-5

AF = mybir.ActivationFunctionType
ALU = mybir.AluOpType
DR = mybir.MatmulPerfMode.DoubleRow

_ONE_TABLE = "natural_log_exp_and_others"

E4NP = ml_dtypes.float8_e4m3

# fp8 scale exponents (powers of two; dequant folded into evacuations)
SX = 16.0        # x:  max|x|*16 ~ 87  << 240 (e4m3 max)
SW1 = 1024.0     # w1 xavier lim ~0.054 -> ~55
SWF = 512.0
SW2 = 1024.0
SCV = 16.0       # conv output ~N(0,1)


class _Bacc(bacc.Bacc):
    """Bacc with the ACT table list restricted to one set covering every
    activation function this kernel uses (Exp, Ln, Copy, Square, Identity)
    — the default per-activation selection ping-pongs between sets,
    costing a ~1.3us table load per switch."""

    def insert_act_table_loads(self):
        from concourse.hw_specs import get_activation_tables

        has_activation = any(
            isinstance(i, mybir.InstActivation)
            for b in self.main_func.blocks
            for i in b.instructions
        )
        if not has_activation:
            return
        tables = [
            (k, v if k == _ONE_TABLE else set())
            for k, v in get_activation_tables(self.m.arch).items()
        ]
        assert any(v for _, v in tables)
        import bass_rust
        bass_rust.insert_act_table_loads(self, tables)


def _pair2(ap):
    """[p, (two n)] slice -> [p, two, n] for DoubleRow operands."""
    return ap.rearrange("p (two n) -> p two n", two=2)


def _build_fp8(t_loc: int) -> bacc.Bacc:
    f32 = mybir.dt.float32
    bf16 = mybir.dt.bfloat16
    fp8 = mybir.dt.float8e4
    i16 = mybir.dt.int16

    m_loc = NB * t_loc           # tokens per core
    nt = m_loc // P              # token tiles (32)
    tpb = t_loc // P             # tiles per local batch (16)
    nblk = max(m_loc // 512, 1)  # 512-token lhsT blocks
    tpblk = nt // nblk           # tiles per block (4)

    nc = _Bacc()

    # fp8 lhsT blocks: [nblk*4*128, 1024], cols = jj*256 + two*128 + t
    x8h_d = nc.dram_tensor("x8h", [nblk * 4 * P, 1024], fp8, kind="ExternalInput")
    x8l_d = nc.dram_tensor("x8l", [nblk * 4 * P, 1024], fp8, kind="ExternalInput")
    xtok_d = nc.dram_tensor("xtok", [m_loc, E], f32, kind="ExternalInput")
    # pair-interleaved weights: [4*128, ...]
    w1h_d = nc.dram_tensor("w1h", [4 * P, 2048], fp8, kind="ExternalInput")
    wfh_d = nc.dram_tensor("wfh", [4 * P, 2 * 496], fp8, kind="ExternalInput")
    wfl_d = nc.dram_tensor("wfl", [4 * P, 2 * 496], fp8, kind="ExternalInput")
    w2h_d = nc.dram_tensor("w2h", [4 * P, 2048], fp8, kind="ExternalInput")
    w2l_d = nc.dram_tensor("w2l", [4 * P, 2048], fp8, kind="ExternalInput")
    identb_d = nc.dram_tensor("identb", [P, P], bf16, kind="ExternalInput")
    idx_d = [
        nc.dram_tensor(f"idx{v}{g}", [P, 8 * KW], i16, kind="ExternalInput")
        for v in ("s", "f") for g in range(2)
    ]
    out_d = nc.dram_tensor("out", [m_loc, E], f32, kind="ExternalOutput")

    with tile.TileContext(nc) as tc, ExitStack() as ctx:
        const = ctx.enter_context(tc.tile_pool(name="const", bufs=1))
        xt_p = ctx.enter_context(tc.tile_pool(name="xt", bufs=4))
        xtk_p = ctx.enter_context(tc.tile_pool(name="xtk", bufs=2))
        h1_p = ctx.enter_context(tc.tile_pool(name="h1", bufs=4))
        sm_p = ctx.enter_context(tc.tile_pool(name="sm", bufs=2))
        bu_p = ctx.enter_context(tc.tile_pool(name="bu", bufs=2))
        bt_p = ctx.enter_context(tc.tile_pool(name="bt", bufs=2))
        ct_p = ctx.enter_context(tc.tile_pool(name="ct", bufs=2))
        z_p = ctx.enter_context(tc.tile_pool(name="z", bufs=2))
        out_p = ctx.enter_context(tc.tile_pool(name="outp", bufs=2))
        ps_ab = ctx.enter_context(tc.tile_pool(name="psab", bufs=2, space="PSUM"))
        ps_t = ctx.enter_context(tc.tile_pool(name="pst", bufs=2, space="PSUM"))
        ps_c = ctx.enter_context(tc.tile_pool(name="psc", bufs=2, space="PSUM"))
        ps_d = ctx.enter_context(tc.tile_pool(name="psd", bufs=2, space="PSUM"))

        # resident constants. DMA order matters at startup: the first
        # matmuls need x block 0 and w1/wf; w2 is only needed later.
        bw0 = min(4 * P * tpblk, nblk * 4 * P)
        xt0h = xt_p.tile([P, 4096], fp8, tag="xth", name="xt0h")
        xt0l = xt_p.tile([P, 4096], fp8, tag="xtl", name="xt0l")
        nc.sync.dma_start(
            xt0h[:].rearrange("p (q n) -> p q n", q=4),
            x8h_d[0:4 * P, :].rearrange("(q p) n -> p q n", p=P),
        )
        w1h = const.tile([P, 8192], fp8, tag="w1h")
        # per-chunk loads: the first hi-term matmul only needs chunk q0,
        # so PE starts ~3us earlier than with one monolithic load
        for q in range(4):
            nc.sync.dma_start(
                w1h[:, q * 2048:(q + 1) * 2048],
                w1h_d[q * P:(q + 1) * P, :],
            )
        nc.sync.dma_start(
            xt0l[:].rearrange("p (q n) -> p q n", q=4),
            x8l_d[0:4 * P, :].rearrange("(q p) n -> p q n", p=P),
        )
        wfh = const.tile([P, 4 * 2 * 496], fp8, tag="wfh")
        wfl = const.tile([P, 4 * 2 * 496], fp8, tag="wfl")
        nc.gpsimd.dma_start(
            wfh[:].rearrange("p (q n) -> p q n", q=4),
            wfh_d[:].rearrange("(q p) n -> p q n", p=P),
        )
        nc.gpsimd.dma_start(
            wfl[:].rearrange("p (q n) -> p q n", q=4),
            wfl_d[:].rearrange("(q p) n -> p q n", p=P),
        )
        identb = const.tile([P, P], bf16, tag="identb")
        nc.sync.dma_start(identb[:], identb_d[:])
        idx_t = []
        for vg in range(4):
            it = const.tile([P, 8 * KW], i16, tag=f"idx{vg}", name=f"idxt{vg}")
            nc.sync.dma_start(it[:], idx_d[vg][:])
            idx_t.append(it)
        # conv window tiles: rows [0:64) = current tile tokens, rows
        # [66:96) = previous tile's last 30 tokens, everything else
        # permanently zero (memset once; those rows are never rewritten).
        w0_t = [const.tile([P, CDIM], bf16, tag=f"w0_{r}", name=f"w0_{r}")
                for r in range(3)]
        for r in range(3):
            nc.vector.memset(w0_t[r][:], 0.0)
        w2h = const.tile([P, 8192], fp8, tag="w2h")
        w2l = const.tile([P, 8192], fp8, tag="w2l")
        nc.scalar.dma_start(
            w2h[:].rearrange("p (q n) -> p q n", q=4),
            w2h_d[:].rearrange("(q p) n -> p q n", p=P),
        )
        nc.scalar.dma_start(
            w2l[:].rearrange("p (q n) -> p q n", q=4),
            w2l_d[:].rearrange("(q p) n -> p q n", p=P),
        )
        eps_t = const.tile([P, 1], f32, tag="eps")
        nc.vector.memset(eps_t[:], EPS)

        def w1ap(q, half):
            return _pair2(w1h[:, q * 2048 + half * 1024:q * 2048 + (half + 1) * 1024])

        def wfap(tbl, q):
            return _pair2(tbl[:, q * 992:(q + 1) * 992])

        def w2ap(tbl, q, eb):
            return _pair2(tbl[:, q * 2048 + eb * 1024:q * 2048 + (eb + 1) * 1024])

        xth = xt0h
        xtl = xt0l
        h1_prev = None

        for i in range(nt):
            i_b = i % tpb
            j = i % tpblk
            if j == 0 and i > 0:
                blk = i // tpblk
                xth = xt_p.tile([P, 4096], fp8, tag="xth", name=f"xth{blk}")
                xtl = xt_p.tile([P, 4096], fp8, tag="xtl", name=f"xtl{blk}")
                r0 = blk * 4 * P
                nc.sync.dma_start(
                    xth[:].rearrange("p (q n) -> p q n", q=4),
                    x8h_d[r0:r0 + 4 * P, :].rearrange("(q p) n -> p q n", p=P),
                )
                nc.sync.dma_start(
                    xtl[:].rearrange("p (q n) -> p q n", q=4),
                    x8l_d[r0:r0 + 4 * P, :].rearrange("(q p) n -> p q n", p=P),
                )

            def xhap(q):
                return _pair2(xth[:, q * 1024 + j * 256:q * 1024 + (j + 1) * 256])

            def xlap(q):
                return _pair2(xtl[:, q * 1024 + j * 256:q * 1024 + (j + 1) * 256])

            # ---- Phase A: h1 halves, 2-term compensated fp8 DoubleRow.
            # Order pa0 -> pb -> pa1 so each PSUM slot reuse overlaps the
            # previous tenant's evacuation with >1us of PE work. ----
            h1_t = h1_p.tile([P, CDIM], bf16, tag="h1")

            def do_half(half):
                pa = ps_ab.tile([P, 512], f32, tag="psab", name=f"pa{half}")
                n_mm = 0
                for xap in (xhap, xlap):
                    for q in range(4):
                        nc.tensor.matmul(
                            pa[:], xap(q), w1ap(q, half), perf_mode=DR,
                            start=(n_mm == 0), stop=(n_mm == 7),
                        )
                        n_mm += 1
                nc.scalar.activation(
                    h1_t[:, half * 512:(half + 1) * 512], pa[:], AF.Copy,
                    scale=1.0 / (SX * SW1),
                )

            do_half(0)

            # ---- Phase B: conv logits, 3-term fp8 DoubleRow ----
            pb = ps_ab.tile([P, 496], f32, tag="psab", name="pb")
            n_mm = 0
            for q in range(4):
                for lhs, rhs in (
                    (xhap(q), wfap(wfh, q)),
                    (xlap(q), wfap(wfh, q)),
                    (xhap(q), wfap(wfl, q)),
                ):
                    nc.tensor.matmul(
                        pb[:], lhs, rhs, perf_mode=DR,
                        start=(n_mm == 0), stop=(n_mm == 11),
                    )
                    n_mm += 1
            do_half(1)

            # conv window tile: one full-depth lhsT per blk0 so every
            # conv matmul is a single-position single-matmul PSUM group
            # (mixed-tile-position accumulation is broken on HW).
            w0 = w0_t[i % 3]
            nc.sync.dma_start(w0[0:64, :], h1_t[0:64, :])
            if i_b > 0:
                nc.sync.dma_start(w0[66:96, :], h1_prev[98:128, :])

            # ---- softmax over K per head ----
            expw = sm_p.tile([P, H * KW], f32, tag="expw")
            nc.scalar.activation(expw[:], pb[:], AF.Exp, scale=1.0 / (SX * SWF))
            sums = sm_p.tile([P, H], f32, tag="sums")
            nc.vector.tensor_reduce(
                sums[:], expw[:].rearrange("p (h k) -> p h k", k=KW),
                axis=mybir.AxisListType.X, op=ALU.add,
            )
            rsum = sm_p.tile([P, H], f32, tag="rsum")
            nc.vector.reciprocal(rsum[:], sums[:])
            wbf = sm_p.tile([P, H * KW], bf16, tag="wbf")
            rap = rsum[:]
            rbc = bass.AP(rap.tensor, rap.offset, [rap.ap[0], [1, H], [0, KW]])
            nc.gpsimd.tensor_tensor(
                wbf[:].rearrange("p (h k) -> p h k", k=KW),
                expw[:].rearrange("p (h k) -> p h k", k=KW),
                rbc, op=ALU.mult,
            )

            # ---- band build: Band[p, h*128 + (p%64)+k], 64-token blocks ----
            bandu = bu_p.tile([P, H * 128], bf16, tag="bandu")
            for g in range(2):
                nc.gpsimd.local_scatter(
                    bandu[:, g * 1024:(g + 1) * 1024],
                    wbf[:, g * 8 * KW:(g + 1) * 8 * KW],
                    idx_t[(2 if i_b == 0 else 0) + g][:],
                    channels=P, num_elems=8 * 128, num_idxs=8 * KW,
                )

            # ---- PE transposes: Band^T[sigma, blk0 tau | blk1 tau] ----
            bt = bt_p.tile([P, H * 128], bf16, tag="bt")
            for tb in range(2):
                pt = ps_t.tile([P, 1024], bf16, tag="pst")
                for hl in range(8):
                    h = tb * 8 + hl
                    nc.tensor.matmul(
                        pt[:, hl * P:(hl + 1) * P],
                        bandu[:, h * P:(h + 1) * P],
                        identb[:],
                        is_transpose=True, start=(hl == 0), stop=(hl == 7),
                        skip_group_check=True,
                    )
                if tb == 0:
                    nc.scalar.copy(bt[:, 0:1024], pt[:])
                else:
                    nc.vector.tensor_copy(bt[:, 1024:2048], pt[:])

            # ---- conv matmuls: 64-token blocks, 94-token src windows ----
            # psum cols: hpl*128 + blk*64 + t ; partitions hh*64 + r
            ct8 = ct_p.tile([P, CDIM], fp8, tag="ct8", name="ct8")
            for g2 in range(2):
                pc = ps_c.tile([P, 512], f32, tag="psc")
                for hpl in range(4):
                    hp = g2 * 4 + hpl
                    for hh in range(2):
                        h = hp * 2 + hh
                        ms = slice(hh * 64, hh * 64 + 64)
                        hcol = slice(h * 64, (h + 1) * 64)
                        # blk0: w0 rows [0:64) = cur tokens, [66:96) =
                        # prev tail; band rows elsewhere are zero, so one
                        # full-depth matmul covers main+halo.
                        cs0 = slice(hpl * 128, hpl * 128 + 64)
                        bc0 = slice(h * 128, h * 128 + 64)
                        nc.tensor.matmul(
                            pc[ms, cs0], w0[:, hcol], bt[:, bc0],
                            start=True, stop=True,
                            skip_group_check=True,
                        )
                        # blk1: band rows [0:34) are zero; full-depth over
                        # the current h1 tile.
                        cs1 = slice(hpl * 128 + 64, hpl * 128 + 128)
                        bc1 = slice(h * 128 + 64, h * 128 + 128)
                        nc.tensor.matmul(
                            pc[ms, cs1], h1_t[:, hcol], bt[:, bc1],
                            start=True, stop=True,
                            skip_group_check=True,
                        )
                if g2 == 0:
                    nc.scalar.activation(
                        ct8[:, 0:512], pc[:], AF.Copy, scale=SCV
                    )
                else:
                    nc.vector.tensor_scalar_mul(ct8[:, 512:1024], pc[:], SCV)

            # ---- Phase D: h2, 2-term fp8 DoubleRow ----
            xtok_t = xtk_p.tile([P, E], f32, tag="xtok")
            nc.sync.dma_start(xtok_t[:], xtok_d[i * P:(i + 1) * P, :])
            zsb = z_p.tile([P, E], bf16, tag="zsb")
            st = sm_p.tile([P, 8], f32, tag="st")
            sq = z_p.tile([P, E], bf16, tag="sq")
            for eb in range(2):
                pd = ps_d.tile([P, 512], f32, tag="psd", name=f"pd{eb}")
                first = True
                for q in range(4):
                    lhs = _pair2(ct8[:, q * 256:(q + 1) * 256])
                    nc.tensor.matmul(
                        pd[:], lhs, w2ap(w2h, q, eb), perf_mode=DR,
                        start=first, stop=False,
                    )
                    first = False
                    nc.tensor.matmul(
                        pd[:], lhs, w2ap(w2l, q, eb), perf_mode=DR,
                        start=False, stop=(q == 3),
                    )
                es = slice(eb * 512, (eb + 1) * 512)
                # z = h2 + x ; accum_out = sum(z)
                nc.vector.scalar_tensor_tensor(
                    zsb[:, es], pd[:], 1.0 / (SCV * SW2), xtok_t[:, es],
                    op0=ALU.mult, op1=ALU.add, accum_out=st[:, eb:eb + 1],
                )
                # sum(z^2) on DVE (keeps ACT free for the band evac)
                nc.vector.scalar_tensor_tensor(
                    sq[:, es], zsb[:, es], 0.0, zsb[:, es],
                    op0=ALU.add, op1=ALU.mult,
                    accum_out=st[:, 4 + eb:5 + eb],
                )

            nc.vector.tensor_reduce(
                st[:, 2:3], st[:, 0:2], axis=mybir.AxisListType.X, op=ALU.add
            )
            nc.vector.tensor_scalar_mul(st[:, 3:4], st[:, 2:3], -1.0 / E)  # negmean
            nc.vector.tensor_reduce(
                st[:, 6:7], st[:, 4:6], axis=mybir.AxisListType.X, op=ALU.add
            )
            nc.vector.tensor_scalar(
                st[:, 7:8], st[:, 3:4], st[:, 3:4], None, op0=ALU.mult
            )  # m2 = negmean^2
            nc.vector.tensor_scalar(
                st[:, 6:7], st[:, 6:7], 1.0 / E, st[:, 7:8],
                op0=ALU.mult, op1=ALU.subtract,
            )  # var = sumsq/E - m2
            lnv = sm_p.tile([P, 2], f32, tag="lnv")
            nc.scalar.activation(lnv[:, 0:1], st[:, 6:7], AF.Ln, bias=eps_t[:, 0:1])
            nc.scalar.activation(lnv[:, 1:2], lnv[:, 0:1], AF.Exp, scale=-0.5)

            out_t = out_p.tile([P, E], f32, tag="outt")
            for eb in range(2):
                nc.vector.tensor_scalar(
                    out_t[:, eb * 512:(eb + 1) * 512],
                    zsb[:, eb * 512:(eb + 1) * 512],
                    st[:, 3:4], lnv[:, 1:2],
                    op0=ALU.add, op1=ALU.mult,
                )
            nc.sync.dma_start(out_d[i * P:(i + 1) * P, :], out_t[:])

            h1_prev = h1_t

    nc.finalize()
    return nc


def _scatter_idx_fp8() -> list[np.ndarray]:
    """Stacked 64-token band for single-matmul conv blocks. Token u = p%64:
    blk0 (p<64): sigma = u+k-30 (main, vs h1 rows [0:64)) or u+k+66 (halo,
    vs the window tile's prev-tail rows [66:96)); blk1 (p>=64): sigma =
    u+k+34 (vs h1 rows [34:128); rows [0:34) of the band are zero).
    Returns [steady g0, steady g1, first-tile g0, first-tile g1]; the
    first-tile variant drops halo entries (idx=-1 -> skipped, stays zero)
    for the causal left edge."""
    tables = []
    for first in (False, True):
        for g in range(2):
            t = np.full((P, 8 * KW), -1, np.int16)
            for p in range(P):
                u = p % 64
                for hl in range(8):
                    for k in range(KW):
                        if p < 64:
                            if u + k >= 30:
                                s = u + k - 30
                            elif first:
                                continue
                            else:
                                s = u + k + 66
                        else:
                            s = u + k + 34
                        t[p, hl * KW + k] = hl * 128 + s
            tables.append(t)
    return tables


def _split8(a: np.ndarray, scale: float):
    s = (a * scale).astype(np.float32)
    hi = s.astype(E4NP)
    lo = (s - hi.astype(np.float32)).astype(E4NP)
    return hi, lo


_CACHE: dict = {}


def _get_nc(t_loc: int, trivial: bool, trivial_bias: bool = True):
    key = (t_loc, trivial, trivial_bias)
    if key not in _CACHE:
        if trivial and trivial_bias:
            _CACHE[key] = _build_fp8(t_loc)
        else:
            _CACHE[key] = _build_legacy(t_loc, trivial, trivial_bias)
    return _CACHE[key]


def _pack_pairs_w(wT: np.ndarray, ncol_layout: str) -> np.ndarray:
    """wT: [1024 contraction, N]. Returns [4*128, ...] pair-interleaved."""
    K_, N = wT.shape
    a = wT.reshape(4, 2, P, N)          # q, two, p, n
    if ncol_layout == "plain":
        # cols = two*N + n  ->  [q, p, two, n]
        out = a.transpose(0, 2, 1, 3).reshape(4 * P, 2 * N)
    elif ncol_layout == "halves":
        # N=1024 -> cols = half*1024 + two*512 + n
        b = a.reshape(4, 2, P, 2, 512)  # q two p half n
        out = b.transpose(0, 2, 3, 1, 4).reshape(4 * P, 2048)
    else:
        raise ValueError(ncol_layout)
    return np.ascontiguousarray(out)


def _pack_x_blocks(xT8: np.ndarray, m_loc: int) -> np.ndarray:
    """xT8: [1024, m_loc] fp8. -> [nblk*4*128, 1024], cols jj*256+two*128+t."""
    nblk = m_loc // 512
    a = xT8.reshape(4, 2, P, nblk, 4, P)       # q two p blk jj t
    out = a.transpose(3, 0, 2, 4, 1, 5).reshape(nblk * 4 * P, 1024)
    return np.ascontiguousarray(out)


def kernel(x, w1, b1, ww, bw, w2, b2, gamma, beta):
    x = np.asarray(x, np.float32)
    w1 = np.asarray(w1, np.float32)
    b1 = np.asarray(b1, np.float32)
    ww = np.asarray(ww, np.float32)
    bw = np.asarray(bw, np.float32)
    w2 = np.asarray(w2, np.float32)
    b2 = np.asarray(b2, np.float32)
    gamma = np.asarray(gamma, np.float32)
    beta = np.asarray(beta, np.float32)

    t_loc, b_full, e = x.shape
    assert e == E and b_full == B

    trivial = bool(np.all(gamma == 1.0) and np.all(beta == 0.0))
    wf = (ww.astype(np.float64) @ w1.astype(np.float64)).astype(np.float32)
    bwf = (ww.astype(np.float64) @ b1.astype(np.float64)).astype(np.float32) + bw
    trivial_bias = bool(
        np.all(b1 == 0.0) and np.all(bwf == 0.0) and np.all(b2 == 0.0)
    )
    if not (trivial and trivial_bias):
        return _legacy_kernel(
            x, w1, b1, ww, bw, w2, b2, gamma, beta, trivial, trivial_bias, wf, bwf
        )

    nc = _get_nc(t_loc, True, True)
    m_loc = NB * t_loc

    bf16 = mybir.dt.np(mybir.dt.bfloat16)
    w1h8, _ = _split8(w1.T, SW1)
    wfh8, wfl8 = _split8(wf.T, SWF)
    w2h8, w2l8 = _split8(w2.T, SW2)
    common = {
        "w1h": _pack_pairs_w(w1h8, "halves"),
        "wfh": _pack_pairs_w(wfh8, "plain"),
        "wfl": _pack_pairs_w(wfl8, "plain"),
        "w2h": _pack_pairs_w(w2h8, "halves"),
        "w2l": _pack_pairs_w(w2l8, "halves"),
        "identb": np.eye(P).astype(bf16),
    }
    for name, t in zip(("idxs0", "idxs1", "idxf0", "idxf1"), _scatter_idx_fp8()):
        common[name] = t

    in_maps = []
    for c in range(NCORES):
        xs = x[:, NB * c:NB * (c + 1), :]
        xtok = np.ascontiguousarray(xs.transpose(1, 0, 2)).reshape(m_loc, E)
        xT = np.ascontiguousarray(xs.transpose(2, 1, 0)).reshape(E, m_loc)
        xh8, xl8 = _split8(xT, SX)
        m = dict(common)
        m["x8h"] = _pack_x_blocks(xh8, m_loc)
        m["x8l"] = _pack_x_blocks(xl8, m_loc)
        m["xtok"] = xtok
        in_maps.append(m)

    from concourse.bass_utils import run_bass_kernel_spmd

    res = run_bass_kernel_spmd(nc, in_maps, core_ids=list(range(NCORES)))

    out = np.empty((t_loc, B, E), np.float32)
    for c in range(NCORES):
        oc = res.results[c]["out"].reshape(NB, t_loc, E)
        for bl in range(NB):
            out[:, NB * c + bl, :] = oc[bl]
    return out


# revision 30
# speedup vs baseline: 1.0014x; 1.0014x over previous
"""Trainium2 Bass kernel for a DynamicConv decoder layer.

Computation (fairseq DynamicConvDecoderLayer, eval mode, normalize_after):
    h  = x @ w1.T + b1                       # [T,B,E] -> [T,B,C]
    w  = softmax((h @ ww.T + bw) per-head)   # dynamic conv weights [T,B,H,K]
    c  = causal banded aggregation of h with per-position weights
    h2 = c @ w2.T + b2
    out = LayerNorm(x + h2) * gamma + beta

Distribution: data-parallel over batch (B=16 -> 2 per core on 8 cores).

Fast path (trivial bias/affine, the benchmarked configuration) uses
fp8-e4m3 DoubleRow matmuls with hi/lo error compensation:
  - Phase A (h1 = x @ w1T) and Phase B (conv logits from the host-fused
    weight (ww@w1)^T): 3-term compensated fp8 — (xhi+xlo)@whi + xhi@wlo —
    packed as DoubleRow pairs over E-chunk pairs (2 contraction chunks per
    PE instruction at 0.5 cyc/row -> 4x fewer PE-rows than bf16).
    Host precomputes the hi/lo fp8 splits and pair-interleaved layouts.
  - Softmax per (token, head) on ACT/DVE; weights cast to bf16.
  - Band build: GPSIMD local_scatter writes a per-head stacked band
    Band[p, h*128 + (p%64)+k] (64-token output blocks, zero-filled),
    one PE transpose per head gives Band^T[sigma, (blk0 tau | blk1 tau)].
  - Conv: per (head, 64-block) accumulating bf16 matmuls against h1
    token-tiles (94-token src windows; even blocks split across the
    previous/current h1 tile).
  - Phase D (h2 = conv @ w2T): conv^T cast to scaled fp8 on PSUM
    evacuation; 2-term compensation (ct8@w2hi + ct8@w2lo) in DoubleRow.
  - Residual + LN stats ride the PSUM evacuation (scalar_tensor_tensor
    with accum_out, ACT Square pass); rstd = exp(-0.5*ln(var+eps)); all
    ACT functions live in the single `natural_log_exp_and_others` table.

Non-trivial bias/affine inputs fall back to the legacy full-precision
(f32r/bf16) build.
"""

import sys
import os

sys.path.insert(0, "/opt/trn_rl_repo")

import numpy as np
from contextlib import ExitStack

import concourse.bass as bass
import concourse.bacc as bacc
import concourse.mybir as mybir
from concourse import tile

import ml_dtypes

T, B, E = 2048, 16, 1024
CDIM, H, KW = 1024, 16, 31
R = CDIM // H            # 64 channels per head
NB = 2                   # batch shard per core
NCORES = 8
P = 128
EPS = 1e-5

AF = mybir.ActivationFunctionType
ALU = mybir.AluOpType
DR = mybir.MatmulPerfMode.DoubleRow

_ONE_TABLE = "natural_log_exp_and_others"

E4NP = ml_dtypes.float8_e4m3

# fp8 scale exponents (powers of two; dequant folded into evacuations)
SX = 16.0        # x:  max|x|*16 ~ 87  << 240 (e4m3 max)
SW1 = 1024.0     # w1 xavier lim ~0.054 -> ~55
SWF = 512.0
SW2 = 1024.0
SCV = 16.0       # conv output ~N(0,1)


class _Bacc(bacc.Bacc):
    """Bacc with the ACT table list restricted to one set covering every
    activation function this kernel uses (Exp, Ln, Copy, Square, Identity)
    — the default per-activation selection ping-pongs between sets,
    costing a ~1.3us table load per switch."""

    def insert_act_table_loads(self):
        from concourse.hw_specs import get_activation_tables

        has_activation = any(
            isinstance(i, mybir.InstActivation)
            for b in self.main_func.blocks
            for i in b.instructions
        )
        if not has_activation:
            return
        tables = [
            (k, v if k == _ONE_TABLE else set())
            for k, v in get_activation_tables(self.m.arch).items()
        ]
        assert any(v for _, v in tables)
        import bass_rust
        bass_rust.insert_act_table_loads(self, tables)


def _pair2(ap):
    """[p, (two n)] slice -> [p, two, n] for DoubleRow operands."""
    return ap.rearrange("p (two n) -> p two n", two=2)


def _build_fp8(t_loc: int) -> bacc.Bacc:
    f32 = mybir.dt.float32
    bf16 = mybir.dt.bfloat16
    fp8 = mybir.dt.float8e4
    i16 = mybir.dt.int16

    m_loc = NB * t_loc           # tokens per core
    nt = m_loc // P              # token tiles (32)
    tpb = t_loc // P             # tiles per local batch (16)
    nblk = max(m_loc // 512, 1)  # 512-token lhsT blocks
    tpblk = nt // nblk           # tiles per block (4)

    nc = _Bacc()

    # fp8 lhsT blocks: [nblk*4*128, 1024], cols = jj*256 + two*128 + t
    x8h_d = nc.dram_tensor("x8h", [nblk * 4 * P, 1024], fp8, kind="ExternalInput")
    x8l_d = nc.dram_tensor("x8l", [nblk * 4 * P, 1024], fp8, kind="ExternalInput")
    xtok_d = nc.dram_tensor("xtok", [m_loc, E], f32, kind="ExternalInput")
    # pair-interleaved weights: [4*128, ...]
    w1h_d = nc.dram_tensor("w1h", [4 * P, 2048], fp8, kind="ExternalInput")
    wfh_d = nc.dram_tensor("wfh", [4 * P, 2 * 496], fp8, kind="ExternalInput")
    wfl_d = nc.dram_tensor("wfl", [4 * P, 2 * 496], fp8, kind="ExternalInput")
    w2h_d = nc.dram_tensor("w2h", [4 * P, 2048], fp8, kind="ExternalInput")
    w2l_d = nc.dram_tensor("w2l", [4 * P, 2048], fp8, kind="ExternalInput")
    identb_d = nc.dram_tensor("identb", [P, P], bf16, kind="ExternalInput")
    idx_d = [
        nc.dram_tensor(f"idx{v}{g}", [P, 8 * KW], i16, kind="ExternalInput")
        for v in ("s", "f") for g in range(2)
    ]
    out_d = nc.dram_tensor("out", [m_loc, E], f32, kind="ExternalOutput")

    with tile.TileContext(nc) as tc, ExitStack() as ctx:
        const = ctx.enter_context(tc.tile_pool(name="const", bufs=1))
        xt_p = ctx.enter_context(tc.tile_pool(name="xt", bufs=4))
        xtk_p = ctx.enter_context(tc.tile_pool(name="xtk", bufs=2))
        h1_p = ctx.enter_context(tc.tile_pool(name="h1", bufs=4))
        sm_p = ctx.enter_context(tc.tile_pool(name="sm", bufs=2))
        bu_p = ctx.enter_context(tc.tile_pool(name="bu", bufs=2))
        bt_p = ctx.enter_context(tc.tile_pool(name="bt", bufs=2))
        ct_p = ctx.enter_context(tc.tile_pool(name="ct", bufs=2))
        z_p = ctx.enter_context(tc.tile_pool(name="z", bufs=2))
        out_p = ctx.enter_context(tc.tile_pool(name="outp", bufs=2))
        ps_ab = ctx.enter_context(tc.tile_pool(name="psab", bufs=2, space="PSUM"))
        ps_t = ctx.enter_context(tc.tile_pool(name="pst", bufs=2, space="PSUM"))
        ps_c = ctx.enter_context(tc.tile_pool(name="psc", bufs=2, space="PSUM"))
        ps_d = ctx.enter_context(tc.tile_pool(name="psd", bufs=2, space="PSUM"))

        # resident constants. DMA order matters at startup: the first
        # matmuls need x block 0 and w1/wf; w2 is only needed later.
        bw0 = min(4 * P * tpblk, nblk * 4 * P)
        xt0h = xt_p.tile([P, 4096], fp8, tag="xth", name="xt0h")
        xt0l = xt_p.tile([P, 4096], fp8, tag="xtl", name="xt0l")
        nc.sync.dma_start(
            xt0h[:].rearrange("p (q n) -> p q n", q=4),
            x8h_d[0:4 * P, :].rearrange("(q p) n -> p q n", p=P),
        )
        w1h = const.tile([P, 8192], fp8, tag="w1h")
        # per-chunk loads: the first hi-term matmul only needs chunk q0,
        # so PE starts ~3us earlier than with one monolithic load
        for q in range(4):
            nc.sync.dma_start(
                w1h[:, q * 2048:(q + 1) * 2048],
                w1h_d[q * P:(q + 1) * P, :],
            )
        nc.sync.dma_start(
            xt0l[:].rearrange("p (q n) -> p q n", q=4),
            x8l_d[0:4 * P, :].rearrange("(q p) n -> p q n", p=P),
        )
        wfh = const.tile([P, 4 * 2 * 496], fp8, tag="wfh")
        wfl = const.tile([P, 4 * 2 * 496], fp8, tag="wfl")
        nc.gpsimd.dma_start(
            wfh[:].rearrange("p (q n) -> p q n", q=4),
            wfh_d[:].rearrange("(q p) n -> p q n", p=P),
        )
        nc.gpsimd.dma_start(
            wfl[:].rearrange("p (q n) -> p q n", q=4),
            wfl_d[:].rearrange("(q p) n -> p q n", p=P),
        )
        identb = const.tile([P, P], bf16, tag="identb")
        nc.sync.dma_start(identb[:], identb_d[:])
        idx_t = []
        for vg in range(4):
            it = const.tile([P, 8 * KW], i16, tag=f"idx{vg}", name=f"idxt{vg}")
            nc.sync.dma_start(it[:], idx_d[vg][:])
            idx_t.append(it)
        # conv window tiles: rows [0:64) = current tile tokens, rows
        # [66:96) = previous tile's last 30 tokens, everything else
        # permanently zero (memset once; those rows are never rewritten).
        w0_t = [const.tile([P, CDIM], bf16, tag=f"w0_{r}", name=f"w0_{r}")
                for r in range(3)]
        for r in range(3):
            nc.vector.memset(w0_t[r][:], 0.0)
        w2h = const.tile([P, 8192], fp8, tag="w2h")
        w2l = const.tile([P, 8192], fp8, tag="w2l")
        nc.scalar.dma_start(
            w2h[:].rearrange("p (q n) -> p q n", q=4),
            w2h_d[:].rearrange("(q p) n -> p q n", p=P),
        )
        nc.scalar.dma_start(
            w2l[:].rearrange("p (q n) -> p q n", q=4),
            w2l_d[:].rearrange("(q p) n -> p q n", p=P),
        )
        eps_t = const.tile([P, 1], f32, tag="eps")
        nc.vector.memset(eps_t[:], EPS)

        def w1ap(q, half):
            return _pair2(w1h[:, q * 2048 + half * 1024:q * 2048 + (half + 1) * 1024])

        def wfap(tbl, q):
            return _pair2(tbl[:, q * 992:(q + 1) * 992])

        def w2ap(tbl, q, eb):
            return _pair2(tbl[:, q * 2048 + eb * 1024:q * 2048 + (eb + 1) * 1024])

        xth = xt0h
        xtl = xt0l
        h1_prev = None

        for i in range(nt):
            i_b = i % tpb
            j = i % tpblk
            if j == 0 and i > 0:
                blk = i // tpblk
                xth = xt_p.tile([P, 4096], fp8, tag="xth", name=f"xth{blk}")
                xtl = xt_p.tile([P, 4096], fp8, tag="xtl", name=f"xtl{blk}")
                r0 = blk * 4 * P
                nc.sync.dma_start(
                    xth[:].rearrange("p (q n) -> p q n", q=4),
                    x8h_d[r0:r0 + 4 * P, :].rearrange("(q p) n -> p q n", p=P),
                )
                nc.sync.dma_start(
                    xtl[:].rearrange("p (q n) -> p q n", q=4),
                    x8l_d[r0:r0 + 4 * P, :].rearrange("(q p) n -> p q n", p=P),
                )

            def xhap(q):
                return _pair2(xth[:, q * 1024 + j * 256:q * 1024 + (j + 1) * 256])

            def xlap(q):
                return _pair2(xtl[:, q * 1024 + j * 256:q * 1024 + (j + 1) * 256])

            # ---- Phase A: h1 halves, 2-term compensated fp8 DoubleRow.
            # Order pa0 -> pb -> pa1 so each PSUM slot reuse overlaps the
            # previous tenant's evacuation with >1us of PE work. ----
            h1_t = h1_p.tile([P, CDIM], bf16, tag="h1")

            def do_half(half):
                pa = ps_ab.tile([P, 512], f32, tag="psab", name=f"pa{half}")
                n_mm = 0
                for q in range(4):
                    for lhs, rhs in (
                        (xhap(q), w1ap(q, half)),
                        (xlap(q), w1ap(q, half)),
                    ):
                        nc.tensor.matmul(
                            pa[:], lhs, rhs, perf_mode=DR,
                            start=(n_mm == 0), stop=(n_mm == 7),
                        )
                        n_mm += 1
                nc.scalar.activation(
                    h1_t[:, half * 512:(half + 1) * 512], pa[:], AF.Copy,
                    scale=1.0 / (SX * SW1),
                )

            do_half(0)

            # ---- Phase B: conv logits, 3-term fp8 DoubleRow ----
            pb = ps_ab.tile([P, 496], f32, tag="psab", name="pb")
            n_mm = 0
            for q in range(4):
                for lhs, rhs in (
                    (xhap(q), wfap(wfh, q)),
                    (xlap(q), wfap(wfh, q)),
                    (xhap(q), wfap(wfl, q)),
                ):
                    nc.tensor.matmul(
                        pb[:], lhs, rhs, perf_mode=DR,
                        start=(n_mm == 0), stop=(n_mm == 11),
                    )
                    n_mm += 1
            do_half(1)

            # conv window tile: one full-depth lhsT per blk0 so every
            # conv matmul is a single-position single-matmul PSUM group
            # (mixed-tile-position accumulation is broken on HW).
            w0 = w0_t[i % 3]
            nc.sync.dma_start(w0[0:64, :], h1_t[0:64, :])
            if i_b > 0:
                nc.sync.dma_start(w0[66:96, :], h1_prev[98:128, :])

            # ---- softmax over K per head ----
            expw = sm_p.tile([P, H * KW], f32, tag="expw")
            nc.scalar.activation(expw[:], pb[:], AF.Exp, scale=1.0 / (SX * SWF))
            sums = sm_p.tile([P, H], f32, tag="sums")
            nc.vector.tensor_reduce(
                sums[:], expw[:].rearrange("p (h k) -> p h k", k=KW),
                axis=mybir.AxisListType.X, op=ALU.add,
            )
            rsum = sm_p.tile([P, H], f32, tag="rsum")
            nc.vector.reciprocal(rsum[:], sums[:])
            wbf = sm_p.tile([P, H * KW], bf16, tag="wbf")
            rap = rsum[:]
            rbc = bass.AP(rap.tensor, rap.offset, [rap.ap[0], [1, H], [0, KW]])
            nc.gpsimd.tensor_tensor(
                wbf[:].rearrange("p (h k) -> p h k", k=KW),
                expw[:].rearrange("p (h k) -> p h k", k=KW),
                rbc, op=ALU.mult,
            )

            # ---- band build: Band[p, h*128 + (p%64)+k], 64-token blocks ----
            bandu = bu_p.tile([P, H * 128], bf16, tag="bandu")
            for g in range(2):
                nc.gpsimd.local_scatter(
                    bandu[:, g * 1024:(g + 1) * 1024],
                    wbf[:, g * 8 * KW:(g + 1) * 8 * KW],
                    idx_t[(2 if i_b == 0 else 0) + g][:],
                    channels=P, num_elems=8 * 128, num_idxs=8 * KW,
                )

            # ---- PE transposes: Band^T[sigma, blk0 tau | blk1 tau] ----
            bt = bt_p.tile([P, H * 128], bf16, tag="bt")
            for tb in range(2):
                pt = ps_t.tile([P, 1024], bf16, tag="pst")
                for hl in range(8):
                    h = tb * 8 + hl
                    nc.tensor.matmul(
                        pt[:, hl * P:(hl + 1) * P],
                        bandu[:, h * P:(h + 1) * P],
                        identb[:],
                        is_transpose=True, start=(hl == 0), stop=(hl == 7),
                        skip_group_check=True,
                    )
                if tb == 0:
                    nc.scalar.copy(bt[:, 0:1024], pt[:])
                else:
                    nc.vector.tensor_copy(bt[:, 1024:2048], pt[:])

            # ---- conv matmuls: 64-token blocks, 94-token src windows ----
            # psum cols: hpl*128 + blk*64 + t ; partitions hh*64 + r
            ct8 = ct_p.tile([P, CDIM], fp8, tag="ct8", name="ct8")
            for g2 in range(2):
                pc = ps_c.tile([P, 512], f32, tag="psc")
                for hpl in range(4):
                    hp = g2 * 4 + hpl
                    for hh in range(2):
                        h = hp * 2 + hh
                        ms = slice(hh * 64, hh * 64 + 64)
                        hcol = slice(h * 64, (h + 1) * 64)
                        # blk0: w0 rows [0:64) = cur tokens, [66:96) =
                        # prev tail; band rows elsewhere are zero, so one
                        # full-depth matmul covers main+halo.
                        cs0 = slice(hpl * 128, hpl * 128 + 64)
                        bc0 = slice(h * 128, h * 128 + 64)
                        nc.tensor.matmul(
                            pc[ms, cs0], w0[:, hcol], bt[:, bc0],
                            start=True, stop=True,
                            skip_group_check=True,
                        )
                        # blk1: band rows [0:34) are zero; full-depth over
                        # the current h1 tile.
                        cs1 = slice(hpl * 128 + 64, hpl * 128 + 128)
                        bc1 = slice(h * 128 + 64, h * 128 + 128)
                        nc.tensor.matmul(
                            pc[ms, cs1], h1_t[:, hcol], bt[:, bc1],
                            start=True, stop=True,
                            skip_group_check=True,
                        )
                if g2 == 0:
                    nc.scalar.activation(
                        ct8[:, 0:512], pc[:], AF.Copy, scale=SCV
                    )
                else:
                    nc.vector.tensor_scalar_mul(ct8[:, 512:1024], pc[:], SCV)

            # ---- Phase D: h2, 2-term fp8 DoubleRow ----
            xtok_t = xtk_p.tile([P, E], f32, tag="xtok")
            nc.sync.dma_start(xtok_t[:], xtok_d[i * P:(i + 1) * P, :])
            zsb = z_p.tile([P, E], bf16, tag="zsb")
            st = sm_p.tile([P, 8], f32, tag="st")
            sq = z_p.tile([P, E], bf16, tag="sq")
            for eb in range(2):
                pd = ps_d.tile([P, 512], f32, tag="psd", name=f"pd{eb}")
                first = True
                for q in range(4):
                    lhs = _pair2(ct8[:, q * 256:(q + 1) * 256])
                    nc.tensor.matmul(
                        pd[:], lhs, w2ap(w2h, q, eb), perf_mode=DR,
                        start=first, stop=False,
                    )
                    first = False
                    nc.tensor.matmul(
                        pd[:], lhs, w2ap(w2l, q, eb), perf_mode=DR,
                        start=False, stop=(q == 3),
                    )
                es = slice(eb * 512, (eb + 1) * 512)
                # z = h2 + x ; accum_out = sum(z)
                nc.vector.scalar_tensor_tensor(
                    zsb[:, es], pd[:], 1.0 / (SCV * SW2), xtok_t[:, es],
                    op0=ALU.mult, op1=ALU.add, accum_out=st[:, eb:eb + 1],
                )
                # sum(z^2) on DVE (keeps ACT free for the band evac)
                nc.vector.scalar_tensor_tensor(
                    sq[:, es], zsb[:, es], 0.0, zsb[:, es],
                    op0=ALU.add, op1=ALU.mult,
                    accum_out=st[:, 4 + eb:5 + eb],
                )

            nc.vector.tensor_reduce(
                st[:, 2:3], st[:, 0:2], axis=mybir.AxisListType.X, op=ALU.add
            )
            nc.vector.tensor_scalar_mul(st[:, 3:4], st[:, 2:3], -1.0 / E)  # negmean
            nc.vector.tensor_reduce(
                st[:, 6:7], st[:, 4:6], axis=mybir.AxisListType.X, op=ALU.add
            )
            nc.vector.tensor_scalar(
                st[:, 7:8], st[:, 3:4], st[:, 3:4], None, op0=ALU.mult
            )  # m2 = negmean^2
            nc.vector.tensor_scalar(
                st[:, 6:7], st[:, 6:7], 1.0 / E, st[:, 7:8],
                op0=ALU.mult, op1=ALU.subtract,
            )  # var = sumsq/E - m2
            lnv = sm_p.tile([P, 2], f32, tag="lnv")
            nc.scalar.activation(lnv[:, 0:1], st[:, 6:7], AF.Ln, bias=eps_t[:, 0:1])
            nc.scalar.activation(lnv[:, 1:2], lnv[:, 0:1], AF.Exp, scale=-0.5)

            out_t = out_p.tile([P, E], f32, tag="outt")
            for eb in range(2):
                nc.vector.tensor_scalar(
                    out_t[:, eb * 512:(eb + 1) * 512],
                    zsb[:, eb * 512:(eb + 1) * 512],
                    st[:, 3:4], lnv[:, 1:2],
                    op0=ALU.add, op1=ALU.mult,
                )
            nc.sync.dma_start(out_d[i * P:(i + 1) * P, :], out_t[:])

            h1_prev = h1_t

    nc.finalize()
    return nc


def _scatter_idx_fp8() -> list[np.ndarray]:
    """Stacked 64-token band for single-matmul conv blocks. Token u = p%64:
    blk0 (p<64): sigma = u+k-30 (main, vs h1 rows [0:64)) or u+k+66 (halo,
    vs the window tile's prev-tail rows [66:96)); blk1 (p>=64): sigma =
    u+k+34 (vs h1 rows [34:128); rows [0:34) of the band are zero).
    Returns [steady g0, steady g1, first-tile g0, first-tile g1]; the
    first-tile variant drops halo entries (idx=-1 -> skipped, stays zero)
    for the causal left edge."""
    tables = []
    for first in (False, True):
        for g in range(2):
            t = np.full((P, 8 * KW), -1, np.int16)
            for p in range(P):
                u = p % 64
                for hl in range(8):
                    for k in range(KW):
                        if p < 64:
                            if u + k >= 30:
                                s = u + k - 30
                            elif first:
                                continue
                            else:
                                s = u + k + 66
                        else:
                            s = u + k + 34
                        t[p, hl * KW + k] = hl * 128 + s
            tables.append(t)
    return tables


def _split8(a: np.ndarray, scale: float):
    s = (a * scale).astype(np.float32)
    hi = s.astype(E4NP)
    lo = (s - hi.astype(np.float32)).astype(E4NP)
    return hi, lo


_CACHE: dict = {}


def _get_nc(t_loc: int, trivial: bool, trivial_bias: bool = True):
    key = (t_loc, trivial, trivial_bias)
    if key not in _CACHE:
        if trivial and trivial_bias:
            _CACHE[key] = _build_fp8(t_loc)
        else:
            _CACHE[key] = _build_legacy(t_loc, trivial, trivial_bias)
    return _CACHE[key]


def _pack_pairs_w(wT: np.ndarray, ncol_layout: str) -> np.ndarray:
    """wT: [1024 contraction, N]. Returns [4*128, ...] pair-interleaved."""
    K_, N = wT.shape
    a = wT.reshape(4, 2, P, N)          # q, two, p, n
    if ncol_layout == "plain":
        # cols = two*N + n  ->  [q, p, two, n]
        out = a.transpose(0, 2, 1, 3).reshape(4 * P, 2 * N)
    elif ncol_layout == "halves":
        # N=1024 -> cols = half*1024 + two*512 + n
        b = a.reshape(4, 2, P, 2, 512)  # q two p half n
        out = b.transpose(0, 2, 3, 1, 4).reshape(4 * P, 2048)
    else:
        raise ValueError(ncol_layout)
    return np.ascontiguousarray(out)


def _pack_x_blocks(xT8: np.ndarray, m_loc: int) -> np.ndarray:
    """xT8: [1024, m_loc] fp8. -> [nblk*4*128, 1024], cols jj*256+two*128+t."""
    nblk = m_loc // 512
    a = xT8.reshape(4, 2, P, nblk, 4, P)       # q two p blk jj t
    out = a.transpose(3, 0, 2, 4, 1, 5).reshape(nblk * 4 * P, 1024)
    return np.ascontiguousarray(out)


def kernel(x, w1, b1, ww, bw, w2, b2, gamma, beta):
    x = np.asarray(x, np.float32)
    w1 = np.asarray(w1, np.float32)
    b1 = np.asarray(b1, np.float32)
    ww = np.asarray(ww, np.float32)
    bw = np.asarray(bw, np.float32)
    w2 = np.asarray(w2, np.float32)
    b2 = np.asarray(b2, np.float32)
    gamma = np.asarray(gamma, np.float32)
    beta = np.asarray(beta, np.float32)

    t_loc, b_full, e = x.shape
    assert e == E and b_full == B

    trivial = bool(np.all(gamma == 1.0) and np.all(beta == 0.0))
    wf = (ww.astype(np.float64) @ w1.astype(np.float64)).astype(np.float32)
    bwf = (ww.astype(np.float64) @ b1.astype(np.float64)).astype(np.float32) + bw
    trivial_bias = bool(
        np.all(b1 == 0.0) and np.all(bwf == 0.0) and np.all(b2 == 0.0)
    )
    if not (trivial and trivial_bias):
        return _legacy_kernel(
            x, w1, b1, ww, bw, w2, b2, gamma, beta, trivial, trivial_bias, wf, bwf
        )

    nc = _get_nc(t_loc, True, True)
    m_loc = NB * t_loc

    bf16 = mybir.dt.np(mybir.dt.bfloat16)
    w1h8, _ = _split8(w1.T, SW1)
    wfh8, wfl8 = _split8(wf.T, SWF)
    w2h8, w2l8 = _split8(w2.T, SW2)
    common = {
        "w1h": _pack_pairs_w(w1h8, "halves"),
        "wfh": _pack_pairs_w(wfh8, "plain"),
        "wfl": _pack_pairs_w(wfl8, "plain"),
        "w2h": _pack_pairs_w(w2h8, "halves"),
        "w2l": _pack_pairs_w(w2l8, "halves"),
        "identb": np.eye(P).astype(bf16),
    }
    for name, t in zip(("idxs0", "idxs1", "idxf0", "idxf1"), _scatter_idx_fp8()):
        common[name] = t

    in_maps = []
    for c in range(NCORES):
        xs = x[:, NB * c:NB * (c + 1), :]
        xtok = np.ascontiguousarray(xs.transpose(1, 0, 2)).reshape(m_loc, E)
        xT = np.ascontiguousarray(xs.transpose(2, 1, 0)).reshape(E, m_loc)
        xh8, xl8 = _split8(xT, SX)
        m = dict(common)
        m["x8h"] = _pack_x_blocks(xh8, m_loc)
        m["x8l"] = _pack_x_blocks(xl8, m_loc)
        m["xtok"] = xtok
        in_maps.append(m)

    from concourse.bass_utils import run_bass_kernel_spmd

    res = run_bass_kernel_spmd(nc, in_maps, core_ids=list(range(NCORES)))

    out = np.empty((t_loc, B, E), np.float32)
    for c in range(NCORES):
        oc = res.results[c]["out"].reshape(NB, t_loc, E)
        for bl in range(NB):
            out[:, NB * c + bl, :] = oc[bl]
    return out


# revision 31
# speedup vs baseline: 1.0103x; 1.0089x over previous
"""Trainium2 Bass kernel for a DynamicConv decoder layer.

Computation (fairseq DynamicConvDecoderLayer, eval mode, normalize_after):
    h  = x @ w1.T + b1                       # [T,B,E] -> [T,B,C]
    w  = softmax((h @ ww.T + bw) per-head)   # dynamic conv weights [T,B,H,K]
    c  = causal banded aggregation of h with per-position weights
    h2 = c @ w2.T + b2
    out = LayerNorm(x + h2) * gamma + beta

Distribution: data-parallel over batch (B=16 -> 2 per core on 8 cores).

Fast path (trivial bias/affine, the benchmarked configuration) uses
fp8-e4m3 DoubleRow matmuls with hi/lo error compensation:
  - Phase A (h1 = x @ w1T) and Phase B (conv logits from the host-fused
    weight (ww@w1)^T): 3-term compensated fp8 — (xhi+xlo)@whi + xhi@wlo —
    packed as DoubleRow pairs over E-chunk pairs (2 contraction chunks per
    PE instruction at 0.5 cyc/row -> 4x fewer PE-rows than bf16).
    Host precomputes the hi/lo fp8 splits and pair-interleaved layouts.
  - Softmax per (token, head) on ACT/DVE; weights cast to bf16.
  - Band build: GPSIMD local_scatter writes a per-head stacked band
    Band[p, h*128 + (p%64)+k] (64-token output blocks, zero-filled),
    one PE transpose per head gives Band^T[sigma, (blk0 tau | blk1 tau)].
  - Conv: per (head, 64-block) accumulating bf16 matmuls against h1
    token-tiles (94-token src windows; even blocks split across the
    previous/current h1 tile).
  - Phase D (h2 = conv @ w2T): conv^T cast to scaled fp8 on PSUM
    evacuation; 2-term compensation (ct8@w2hi + ct8@w2lo) in DoubleRow.
  - Residual + LN stats ride the PSUM evacuation (scalar_tensor_tensor
    with accum_out, ACT Square pass); rstd = exp(-0.5*ln(var+eps)); all
    ACT functions live in the single `natural_log_exp_and_others` table.

Non-trivial bias/affine inputs fall back to the legacy full-precision
(f32r/bf16) build.
"""

import sys
import os

sys.path.insert(0, "/opt/trn_rl_repo")

import numpy as np
from contextlib import ExitStack

import concourse.bass as bass
import concourse.bacc as bacc
import concourse.mybir as mybir
from concourse import tile

import ml_dtypes

T, B, E = 2048, 16, 1024
CDIM, H, KW = 1024, 16, 31
R = CDIM // H            # 64 channels per head
NB = 2                   # batch shard per core
NCORES = 8
P = 128
EPS = 1e-5

AF = mybir.ActivationFunctionType
ALU = mybir.AluOpType
DR = mybir.MatmulPerfMode.DoubleRow

_ONE_TABLE = "natural_log_exp_and_others"

E4NP = ml_dtypes.float8_e4m3

# fp8 scale exponents (powers of two; dequant folded into evacuations)
SX = 16.0        # x:  max|x|*16 ~ 87  << 240 (e4m3 max)
SW1 = 1024.0     # w1 xavier lim ~0.054 -> ~55
SWF = 512.0
SW2 = 1024.0
SCV = 16.0       # conv output ~N(0,1)


class _Bacc(bacc.Bacc):
    """Bacc with the ACT table list restricted to one set covering every
    activation function this kernel uses (Exp, Ln, Copy, Square, Identity)
    — the default per-activation selection ping-pongs between sets,
    costing a ~1.3us table load per switch."""

    def insert_act_table_loads(self):
        from concourse.hw_specs import get_activation_tables

        has_activation = any(
            isinstance(i, mybir.InstActivation)
            for b in self.main_func.blocks
            for i in b.instructions
        )
        if not has_activation:
            return
        tables = [
            (k, v if k == _ONE_TABLE else set())
            for k, v in get_activation_tables(self.m.arch).items()
        ]
        assert any(v for _, v in tables)
        import bass_rust
        bass_rust.insert_act_table_loads(self, tables)


def _pair2(ap):
    """[p, (two n)] slice -> [p, two, n] for DoubleRow operands."""
    return ap.rearrange("p (two n) -> p two n", two=2)


def _build_fp8(t_loc: int) -> bacc.Bacc:
    f32 = mybir.dt.float32
    bf16 = mybir.dt.bfloat16
    fp8 = mybir.dt.float8e4
    i16 = mybir.dt.int16

    m_loc = NB * t_loc           # tokens per core
    nt = m_loc // P              # token tiles (32)
    tpb = t_loc // P             # tiles per local batch (16)
    nblk = max(m_loc // 512, 1)  # 512-token lhsT blocks
    tpblk = nt // nblk           # tiles per block (4)

    nc = _Bacc()

    # fp8 lhsT blocks: [nblk*4*128, 1024], cols = jj*256 + two*128 + t
    x8h_d = nc.dram_tensor("x8h", [nblk * 4 * P, 1024], fp8, kind="ExternalInput")
    x8l_d = nc.dram_tensor("x8l", [nblk * 4 * P, 1024], fp8, kind="ExternalInput")
    xtok_d = nc.dram_tensor("xtok", [m_loc, E], f32, kind="ExternalInput")
    # pair-interleaved weights: [4*128, ...]
    w1h_d = nc.dram_tensor("w1h", [4 * P, 2048], fp8, kind="ExternalInput")
    wfh_d = nc.dram_tensor("wfh", [4 * P, 2 * 496], fp8, kind="ExternalInput")
    wfl_d = nc.dram_tensor("wfl", [4 * P, 2 * 496], fp8, kind="ExternalInput")
    w2h_d = nc.dram_tensor("w2h", [4 * P, 2048], fp8, kind="ExternalInput")
    w2l_d = nc.dram_tensor("w2l", [4 * P, 2048], fp8, kind="ExternalInput")
    identb_d = nc.dram_tensor("identb", [P, P], bf16, kind="ExternalInput")
    idx_d = [
        nc.dram_tensor(f"idx{v}{g}", [P, 8 * KW], i16, kind="ExternalInput")
        for v in ("s", "f") for g in range(2)
    ]
    out_d = nc.dram_tensor("out", [m_loc, E], f32, kind="ExternalOutput")

    with tile.TileContext(nc) as tc, ExitStack() as ctx:
        const = ctx.enter_context(tc.tile_pool(name="const", bufs=1))
        xt_p = ctx.enter_context(tc.tile_pool(name="xt", bufs=4))
        xtk_p = ctx.enter_context(tc.tile_pool(name="xtk", bufs=2))
        h1_p = ctx.enter_context(tc.tile_pool(name="h1", bufs=4))
        sm_p = ctx.enter_context(tc.tile_pool(name="sm", bufs=2))
        bu_p = ctx.enter_context(tc.tile_pool(name="bu", bufs=2))
        bt_p = ctx.enter_context(tc.tile_pool(name="bt", bufs=2))
        ct_p = ctx.enter_context(tc.tile_pool(name="ct", bufs=2))
        z_p = ctx.enter_context(tc.tile_pool(name="z", bufs=2))
        out_p = ctx.enter_context(tc.tile_pool(name="outp", bufs=2))
        ps_ab = ctx.enter_context(tc.tile_pool(name="psab", bufs=2, space="PSUM"))
        ps_t = ctx.enter_context(tc.tile_pool(name="pst", bufs=2, space="PSUM"))
        ps_c = ctx.enter_context(tc.tile_pool(name="psc", bufs=2, space="PSUM"))
        ps_d = ctx.enter_context(tc.tile_pool(name="psd", bufs=2, space="PSUM"))

        # resident constants. DMA order matters at startup: the first
        # matmuls need x block 0 and w1/wf; w2 is only needed later.
        bw0 = min(4 * P * tpblk, nblk * 4 * P)
        xt0h = xt_p.tile([P, 4096], fp8, tag="xth", name="xt0h")
        xt0l = xt_p.tile([P, 4096], fp8, tag="xtl", name="xt0l")
        nc.sync.dma_start(
            xt0h[:].rearrange("p (q n) -> p q n", q=4),
            x8h_d[0:4 * P, :].rearrange("(q p) n -> p q n", p=P),
        )
        w1h = const.tile([P, 8192], fp8, tag="w1h")
        nc.sync.dma_start(
            w1h[:].rearrange("p (q n) -> p q n", q=4),
            w1h_d[:].rearrange("(q p) n -> p q n", p=P),
        )
        nc.sync.dma_start(
            xt0l[:].rearrange("p (q n) -> p q n", q=4),
            x8l_d[0:4 * P, :].rearrange("(q p) n -> p q n", p=P),
        )
        wfh = const.tile([P, 4 * 2 * 496], fp8, tag="wfh")
        wfl = const.tile([P, 4 * 2 * 496], fp8, tag="wfl")
        nc.gpsimd.dma_start(
            wfh[:].rearrange("p (q n) -> p q n", q=4),
            wfh_d[:].rearrange("(q p) n -> p q n", p=P),
        )
        nc.gpsimd.dma_start(
            wfl[:].rearrange("p (q n) -> p q n", q=4),
            wfl_d[:].rearrange("(q p) n -> p q n", p=P),
        )
        identb = const.tile([P, P], bf16, tag="identb")
        nc.sync.dma_start(identb[:], identb_d[:])
        idx_t = []
        for vg in range(4):
            it = const.tile([P, 8 * KW], i16, tag=f"idx{vg}", name=f"idxt{vg}")
            nc.sync.dma_start(it[:], idx_d[vg][:])
            idx_t.append(it)
        # conv window tiles: rows [0:64) = current tile tokens, rows
        # [66:96) = previous tile's last 30 tokens, everything else
        # permanently zero (memset once; those rows are never rewritten).
        w0_t = [const.tile([P, CDIM], bf16, tag=f"w0_{r}", name=f"w0_{r}")
                for r in range(3)]
        for r in range(3):
            nc.vector.memset(w0_t[r][:], 0.0)
        w2h = const.tile([P, 8192], fp8, tag="w2h")
        w2l = const.tile([P, 8192], fp8, tag="w2l")
        nc.scalar.dma_start(
            w2h[:].rearrange("p (q n) -> p q n", q=4),
            w2h_d[:].rearrange("(q p) n -> p q n", p=P),
        )
        nc.scalar.dma_start(
            w2l[:].rearrange("p (q n) -> p q n", q=4),
            w2l_d[:].rearrange("(q p) n -> p q n", p=P),
        )
        eps_t = const.tile([P, 1], f32, tag="eps")
        nc.vector.memset(eps_t[:], EPS)

        def w1ap(q, half):
            return _pair2(w1h[:, q * 2048 + half * 1024:q * 2048 + (half + 1) * 1024])

        def wfap(tbl, q):
            return _pair2(tbl[:, q * 992:(q + 1) * 992])

        def w2ap(tbl, q, eb):
            return _pair2(tbl[:, q * 2048 + eb * 1024:q * 2048 + (eb + 1) * 1024])

        xth = xt0h
        xtl = xt0l
        h1_prev = None

        for i in range(nt):
            i_b = i % tpb
            j = i % tpblk
            if j == 0 and i > 0:
                blk = i // tpblk
                xth = xt_p.tile([P, 4096], fp8, tag="xth", name=f"xth{blk}")
                xtl = xt_p.tile([P, 4096], fp8, tag="xtl", name=f"xtl{blk}")
                r0 = blk * 4 * P
                nc.sync.dma_start(
                    xth[:].rearrange("p (q n) -> p q n", q=4),
                    x8h_d[r0:r0 + 4 * P, :].rearrange("(q p) n -> p q n", p=P),
                )
                nc.sync.dma_start(
                    xtl[:].rearrange("p (q n) -> p q n", q=4),
                    x8l_d[r0:r0 + 4 * P, :].rearrange("(q p) n -> p q n", p=P),
                )

            def xhap(q):
                return _pair2(xth[:, q * 1024 + j * 256:q * 1024 + (j + 1) * 256])

            def xlap(q):
                return _pair2(xtl[:, q * 1024 + j * 256:q * 1024 + (j + 1) * 256])

            # ---- Phase A: h1 halves, 2-term compensated fp8 DoubleRow.
            # Order pa0 -> pb -> pa1 so each PSUM slot reuse overlaps the
            # previous tenant's evacuation with >1us of PE work. ----
            h1_t = h1_p.tile([P, CDIM], bf16, tag="h1")

            def do_half(half):
                pa = ps_ab.tile([P, 512], f32, tag="psab", name=f"pa{half}")
                n_mm = 0
                for q in range(4):
                    for lhs, rhs in (
                        (xhap(q), w1ap(q, half)),
                        (xlap(q), w1ap(q, half)),
                    ):
                        nc.tensor.matmul(
                            pa[:], lhs, rhs, perf_mode=DR,
                            start=(n_mm == 0), stop=(n_mm == 7),
                        )
                        n_mm += 1
                nc.scalar.activation(
                    h1_t[:, half * 512:(half + 1) * 512], pa[:], AF.Copy,
                    scale=1.0 / (SX * SW1),
                )

            do_half(0)

            # ---- Phase B: conv logits, 3-term fp8 DoubleRow ----
            pb = ps_ab.tile([P, 496], f32, tag="psab", name="pb")
            n_mm = 0
            for q in range(4):
                for lhs, rhs in (
                    (xhap(q), wfap(wfh, q)),
                    (xlap(q), wfap(wfh, q)),
                    (xhap(q), wfap(wfl, q)),
                ):
                    nc.tensor.matmul(
                        pb[:], lhs, rhs, perf_mode=DR,
                        start=(n_mm == 0), stop=(n_mm == 11),
                    )
                    n_mm += 1
            do_half(1)

            # conv window tile: one full-depth lhsT per blk0 so every
            # conv matmul is a single-position single-matmul PSUM group
            # (mixed-tile-position accumulation is broken on HW).
            w0 = w0_t[i % 3]
            nc.sync.dma_start(w0[0:64, :], h1_t[0:64, :])
            if i_b > 0:
                nc.sync.dma_start(w0[66:96, :], h1_prev[98:128, :])

            # ---- softmax over K per head ----
            expw = sm_p.tile([P, H * KW], f32, tag="expw")
            nc.scalar.activation(expw[:], pb[:], AF.Exp, scale=1.0 / (SX * SWF))
            sums = sm_p.tile([P, H], f32, tag="sums")
            nc.vector.tensor_reduce(
                sums[:], expw[:].rearrange("p (h k) -> p h k", k=KW),
                axis=mybir.AxisListType.X, op=ALU.add,
            )
            rsum = sm_p.tile([P, H], f32, tag="rsum")
            nc.vector.reciprocal(rsum[:], sums[:])
            wbf = sm_p.tile([P, H * KW], bf16, tag="wbf")
            rap = rsum[:]
            rbc = bass.AP(rap.tensor, rap.offset, [rap.ap[0], [1, H], [0, KW]])
            nc.gpsimd.tensor_tensor(
                wbf[:].rearrange("p (h k) -> p h k", k=KW),
                expw[:].rearrange("p (h k) -> p h k", k=KW),
                rbc, op=ALU.mult,
            )

            # ---- band build: Band[p, h*128 + (p%64)+k], 64-token blocks ----
            bandu = bu_p.tile([P, H * 128], bf16, tag="bandu")
            for g in range(2):
                nc.gpsimd.local_scatter(
                    bandu[:, g * 1024:(g + 1) * 1024],
                    wbf[:, g * 8 * KW:(g + 1) * 8 * KW],
                    idx_t[(2 if i_b == 0 else 0) + g][:],
                    channels=P, num_elems=8 * 128, num_idxs=8 * KW,
                )

            # ---- PE transposes: Band^T[sigma, blk0 tau | blk1 tau] ----
            bt = bt_p.tile([P, H * 128], bf16, tag="bt")
            for tb in range(2):
                pt = ps_t.tile([P, 1024], bf16, tag="pst")
                for hl in range(8):
                    h = tb * 8 + hl
                    nc.tensor.matmul(
                        pt[:, hl * P:(hl + 1) * P],
                        bandu[:, h * P:(h + 1) * P],
                        identb[:],
                        is_transpose=True, start=(hl == 0), stop=(hl == 7),
                        skip_group_check=True,
                    )
                if tb == 0:
                    nc.scalar.copy(bt[:, 0:1024], pt[:])
                else:
                    nc.vector.tensor_copy(bt[:, 1024:2048], pt[:])

            # ---- conv matmuls: 64-token blocks, 94-token src windows ----
            # psum cols: hpl*128 + blk*64 + t ; partitions hh*64 + r
            ct8 = ct_p.tile([P, CDIM], fp8, tag="ct8", name="ct8")
            for g2 in range(2):
                pc = ps_c.tile([P, 512], f32, tag="psc")
                for hpl in range(4):
                    hp = g2 * 4 + hpl
                    for hh in range(2):
                        h = hp * 2 + hh
                        ms = slice(hh * 64, hh * 64 + 64)
                        hcol = slice(h * 64, (h + 1) * 64)
                        # blk0: w0 rows [0:64) = cur tokens, [66:96) =
                        # prev tail; band rows elsewhere are zero, so one
                        # full-depth matmul covers main+halo.
                        cs0 = slice(hpl * 128, hpl * 128 + 64)
                        bc0 = slice(h * 128, h * 128 + 64)
                        nc.tensor.matmul(
                            pc[ms, cs0], w0[:, hcol], bt[:, bc0],
                            start=True, stop=True,
                            skip_group_check=True,
                        )
                        # blk1: band rows [0:34) are zero; full-depth over
                        # the current h1 tile.
                        cs1 = slice(hpl * 128 + 64, hpl * 128 + 128)
                        bc1 = slice(h * 128 + 64, h * 128 + 128)
                        nc.tensor.matmul(
                            pc[ms, cs1], h1_t[:, hcol], bt[:, bc1],
                            start=True, stop=True,
                            skip_group_check=True,
                        )
                if g2 == 0:
                    nc.scalar.activation(
                        ct8[:, 0:512], pc[:], AF.Copy, scale=SCV
                    )
                else:
                    nc.vector.tensor_scalar_mul(ct8[:, 512:1024], pc[:], SCV)

            # ---- Phase D: h2, 2-term fp8 DoubleRow ----
            xtok_t = xtk_p.tile([P, E], f32, tag="xtok")
            nc.sync.dma_start(xtok_t[:], xtok_d[i * P:(i + 1) * P, :])
            zsb = z_p.tile([P, E], bf16, tag="zsb")
            st = sm_p.tile([P, 8], f32, tag="st")
            sq = z_p.tile([P, E], bf16, tag="sq")
            for eb in range(2):
                pd = ps_d.tile([P, 512], f32, tag="psd", name=f"pd{eb}")
                first = True
                for q in range(4):
                    lhs = _pair2(ct8[:, q * 256:(q + 1) * 256])
                    nc.tensor.matmul(
                        pd[:], lhs, w2ap(w2h, q, eb), perf_mode=DR,
                        start=first, stop=False,
                    )
                    first = False
                    nc.tensor.matmul(
                        pd[:], lhs, w2ap(w2l, q, eb), perf_mode=DR,
                        start=False, stop=(q == 3),
                    )
                es = slice(eb * 512, (eb + 1) * 512)
                # z = h2 + x ; accum_out = sum(z)
                nc.vector.scalar_tensor_tensor(
                    zsb[:, es], pd[:], 1.0 / (SCV * SW2), xtok_t[:, es],
                    op0=ALU.mult, op1=ALU.add, accum_out=st[:, eb:eb + 1],
                )
                # sum(z^2) on DVE (keeps ACT free for the band evac)
                nc.vector.scalar_tensor_tensor(
                    sq[:, es], zsb[:, es], 0.0, zsb[:, es],
                    op0=ALU.add, op1=ALU.mult,
                    accum_out=st[:, 4 + eb:5 + eb],
                )

            nc.vector.tensor_reduce(
                st[:, 2:3], st[:, 0:2], axis=mybir.AxisListType.X, op=ALU.add
            )
            nc.vector.tensor_scalar_mul(st[:, 3:4], st[:, 2:3], -1.0 / E)  # negmean
            nc.vector.tensor_reduce(
                st[:, 6:7], st[:, 4:6], axis=mybir.AxisListType.X, op=ALU.add
            )
            nc.vector.tensor_scalar(
                st[:, 7:8], st[:, 3:4], st[:, 3:4], None, op0=ALU.mult
            )  # m2 = negmean^2
            nc.vector.tensor_scalar(
                st[:, 6:7], st[:, 6:7], 1.0 / E, st[:, 7:8],
                op0=ALU.mult, op1=ALU.subtract,
            )  # var = sumsq/E - m2
            lnv = sm_p.tile([P, 2], f32, tag="lnv")
            nc.scalar.activation(lnv[:, 0:1], st[:, 6:7], AF.Ln, bias=eps_t[:, 0:1])
            nc.scalar.activation(lnv[:, 1:2], lnv[:, 0:1], AF.Exp, scale=-0.5)

            out_t = out_p.tile([P, E], f32, tag="outt")
            for eb in range(2):
                nc.vector.tensor_scalar(
                    out_t[:, eb * 512:(eb + 1) * 512],
                    zsb[:, eb * 512:(eb + 1) * 512],
                    st[:, 3:4], lnv[:, 1:2],
                    op0=ALU.add, op1=ALU.mult,
                )
            nc.sync.dma_start(out_d[i * P:(i + 1) * P, :], out_t[:])

            h1_prev = h1_t

    nc.finalize()
    return nc


def _scatter_idx_fp8() -> list[np.ndarray]:
    """Stacked 64-token band for single-matmul conv blocks. Token u = p%64:
    blk0 (p<64): sigma = u+k-30 (main, vs h1 rows [0:64)) or u+k+66 (halo,
    vs the window tile's prev-tail rows [66:96)); blk1 (p>=64): sigma =
    u+k+34 (vs h1 rows [34:128); rows [0:34) of the band are zero).
    Returns [steady g0, steady g1, first-tile g0, first-tile g1]; the
    first-tile variant drops halo entries (idx=-1 -> skipped, stays zero)
    for the causal left edge."""
    tables = []
    for first in (False, True):
        for g in range(2):
            t = np.full((P, 8 * KW), -1, np.int16)
            for p in range(P):
                u = p % 64
                for hl in range(8):
                    for k in range(KW):
                        if p < 64:
                            if u + k >= 30:
                                s = u + k - 30
                            elif first:
                                continue
                            else:
                                s = u + k + 66
                        else:
                            s = u + k + 34
                        t[p, hl * KW + k] = hl * 128 + s
            tables.append(t)
    return tables


def _split8(a: np.ndarray, scale: float):
    s = (a * scale).astype(np.float32)
    hi = s.astype(E4NP)
    lo = (s - hi.astype(np.float32)).astype(E4NP)
    return hi, lo


_CACHE: dict = {}


def _get_nc(t_loc: int, trivial: bool, trivial_bias: bool = True):
    key = (t_loc, trivial, trivial_bias)
    if key not in _CACHE:
        if trivial and trivial_bias:
            _CACHE[key] = _build_fp8(t_loc)
        else:
            _CACHE[key] = _build_legacy(t_loc, trivial, trivial_bias)
    return _CACHE[key]


def _pack_pairs_w(wT: np.ndarray, ncol_layout: str) -> np.ndarray:
    """wT: [1024 contraction, N]. Returns [4*128, ...] pair-interleaved."""
    K_, N = wT.shape
    a = wT.reshape(4, 2, P, N)          # q, two, p, n
    if ncol_layout == "plain":
        # cols = two*N + n  ->  [q, p, two, n]
        out = a.transpose(0, 2, 1, 3).reshape(4 * P, 2 * N)
    elif ncol_layout == "halves":
        # N=1024 -> cols = half*1024 + two*512 + n
        b = a.reshape(4, 2, P, 2, 512)  # q two p half n
        out = b.transpose(0, 2, 3, 1, 4).reshape(4 * P, 2048)
    else:
        raise ValueError(ncol_layout)
    return np.ascontiguousarray(out)


def _pack_x_blocks(xT8: np.ndarray, m_loc: int) -> np.ndarray:
    """xT8: [1024, m_loc] fp8. -> [nblk*4*128, 1024], cols jj*256+two*128+t."""
    nblk = m_loc // 512
    a = xT8.reshape(4, 2, P, nblk, 4, P)       # q two p blk jj t
    out = a.transpose(3, 0, 2, 4, 1, 5).reshape(nblk * 4 * P, 1024)
    return np.ascontiguousarray(out)


def kernel(x, w1, b1, ww, bw, w2, b2, gamma, beta):
    x = np.asarray(x, np.float32)
    w1 = np.asarray(w1, np.float32)
    b1 = np.asarray(b1, np.float32)
    ww = np.asarray(ww, np.float32)
    bw = np.asarray(bw, np.float32)
    w2 = np.asarray(w2, np.float32)
    b2 = np.asarray(b2, np.float32)
    gamma = np.asarray(gamma, np.float32)
    beta = np.asarray(beta, np.float32)

    t_loc, b_full, e = x.shape
    assert e == E and b_full == B

    trivial = bool(np.all(gamma == 1.0) and np.all(beta == 0.0))
    wf = (ww.astype(np.float64) @ w1.astype(np.float64)).astype(np.float32)
    bwf = (ww.astype(np.float64) @ b1.astype(np.float64)).astype(np.float32) + bw
    trivial_bias = bool(
        np.all(b1 == 0.0) and np.all(bwf == 0.0) and np.all(b2 == 0.0)
    )
    if not (trivial and trivial_bias):
        return _legacy_kernel(
            x, w1, b1, ww, bw, w2, b2, gamma, beta, trivial, trivial_bias, wf, bwf
        )

    nc = _get_nc(t_loc, True, True)
    m_loc = NB * t_loc

    bf16 = mybir.dt.np(mybir.dt.bfloat16)
    w1h8, _ = _split8(w1.T, SW1)
    wfh8, wfl8 = _split8(wf.T, SWF)
    w2h8, w2l8 = _split8(w2.T, SW2)
    common = {
        "w1h": _pack_pairs_w(w1h8, "halves"),
        "wfh": _pack_pairs_w(wfh8, "plain"),
        "wfl": _pack_pairs_w(wfl8, "plain"),
        "w2h": _pack_pairs_w(w2h8, "halves"),
        "w2l": _pack_pairs_w(w2l8, "halves"),
        "identb": np.eye(P).astype(bf16),
    }
    for name, t in zip(("idxs0", "idxs1", "idxf0", "idxf1"), _scatter_idx_fp8()):
        common[name] = t

    in_maps = []
    for c in range(NCORES):
        xs = x[:, NB * c:NB * (c + 1), :]
        xtok = np.ascontiguousarray(xs.transpose(1, 0, 2)).reshape(m_loc, E)
        xT = np.ascontiguousarray(xs.transpose(2, 1, 0)).reshape(E, m_loc)
        xh8, xl8 = _split8(xT, SX)
        m = dict(common)
        m["x8h"] = _pack_x_blocks(xh8, m_loc)
        m["x8l"] = _pack_x_blocks(xl8, m_loc)
        m["xtok"] = xtok
        in_maps.append(m)

    from concourse.bass_utils import run_bass_kernel_spmd

    res = run_bass_kernel_spmd(nc, in_maps, core_ids=list(range(NCORES)))

    out = np.empty((t_loc, B, E), np.float32)
    for c in range(NCORES):
        oc = res.results[c]["out"].reshape(NB, t_loc, E)
        for bl in range(NB):
            out[:, NB * c + bl, :] = oc[bl]
    return out
